# revision 1
# baseline (speedup 1.0000x reference)
"""Trainium2 Bass kernel for a 2-layer GCN + global mean pool + MLP head.

Distribution (8 NeuronCores): edge-parallel. Edges (plus one self-loop per
node) are sharded across cores as part of input distribution; each core
gathers node-table rows by src (dma_gather) and scatter-adds them by dst
(dma_scatter_add with SDMA CCE f32 add) into per-core partial accumulators;
node-boundary partial sums are combined with AllReduce. Small parameters are
replicated.

Math: with c = rsqrt(deg) (deg counts in-edges incl. the self loop), each
GCN layer is  h' = relu(c * (sum_{u->v} t[u]) + b)  with  t = c * (h @ W).
The layer-2 weight multiply commutes with the edge sum, so the second edge
pass scatters u1 = c * h1 rows and W2 is applied after the reduce. Column 32
of the u1 rows carries the constant 1, so acc2[:,32] reproduces deg and the
final phase is self-contained per gathered row.

Race-freedom: duplicate scatter destinations within one dma_scatter_add and
across concurrently-running ones are not accumulated correctly by the DMA
engines, so the host deals each (src-chunk, dst-chunk) edge segment into
bins with unique dst (rank-within-dst dealing), and all scatter instructions
that target the same dst-chunk accumulator are chained with explicit deps.
"""

import numpy as np

import concourse.bacc as bacc
import concourse.mybir as mybir
import concourse.tile as tile
from concourse.bass_utils import run_bass_kernel_spmd
from bass_rust import add_dep_helper

# ---- problem geometry (hardcoded per task contract) ----
N = 100000
E = 1000000
G = 256
NTYPES = 200
EMB = 64            # embedding dim; also the 256B table row width (64 f32)
HID = 32
C1 = 16
NCORES = 8

CH_REAL = 25600     # real node rows per chunk (int16-addressable)
CH_PAD = 32768      # chunk stride (16 * 2048)
NCHUNK = 4
NROW = NCHUNK * CH_PAD            # 131072 padded rows
NDENSE = NROW // 2048             # 64 dense chunks
DCH_PER = CH_PAD // 2048          # 16 dense chunks per node chunk
FSLICE = CH_PAD // NCORES         # 4096 rows per core per chunk (final phase)
F32 = mybir.dt.float32
I16 = mybir.dt.int16
I32 = mybir.dt.int32

MAX_WAITS = 1


def _split_sync_waits(nc):
    """walrus TPB codegen encodes at most one sync-wait per instruction;
    split longer wait lists into preceding same-engine nops."""
    n = 0
    for f in nc.m.functions:
        for blk in f.blocks:
            il = blk.instructions
            i = 0
            while i < len(il):
                ins = il[i]
                si = ins.sync_info
                if si is not None and si.on_wait and len(si.on_wait) > MAX_WAITS:
                    w = list(si.on_wait)
                    si.on_wait = w[-MAX_WAITS:]
                    ex = w[:-MAX_WAITS]
                    nops = []
                    for k in range(0, len(ex), MAX_WAITS):
                        p = mybir.InstNoOp(name=f"Wsplit-{n}-{k}")
                        p.engine = ins.engine
                        p.sync_info = mybir.SyncInfo(on_wait=ex[k:k + MAX_WAITS], on_update=[])
                        nops.append(p)
                    for j, p in enumerate(nops):
                        il.insert(i + j, p)
                    i += len(nops)
                    n += 1
                i += 1
    return n


def _shard_edges(src, dst):
    """Shard edges + self loops across cores; group per (src_chunk,
    dst_chunk); deal into unique-dst bins; pad bins to a structure common to
    all cores (the SPMD program is shared). Returns (plan, per_core) where
    plan = [(s, d, seg_lo, seg_hi, [(bin_lo, bin_hi), ...])] in slot units
    and per_core = list of dicts with int16 gidx/sidx flat slot arrays.
    """
    selfn = np.arange(N, dtype=np.int64)
    e_core = np.arange(E) % NCORES
    s_core = selfn % NCORES
    raw = []   # raw[c][seg] = (ss, dd, rank, ks)
    for c in range(NCORES):
        s = np.concatenate([src[e_core == c], selfn[s_core == c]])
        d = np.concatenate([dst[e_core == c], selfn[s_core == c]])
        seg_key = (s // CH_REAL) * NCHUNK + (d // CH_REAL)
        segs = {}
        for seg in range(NCHUNK * NCHUNK):
            m = seg_key == seg
            ss, dd = s[m], d[m]
            if len(dd):
                do = np.argsort(dd, kind="stable")
                ss, dd = ss[do], dd[do]
                grp = np.flatnonzero(np.r_[True, dd[1:] != dd[:-1]])
                rank = np.arange(len(dd)) - np.repeat(grp, np.diff(np.r_[grp, len(dd)]))
                ks = int(rank.max()) + 1
            else:
                rank, ks = np.zeros(0, np.int64), 0
            segs[seg] = (ss, dd, rank, ks)
        raw.append(segs)

    # common bin sizes (padded to 128 slots)
    plan = []
    slot = 0
    binsizes = {}
    for seg in range(NCHUNK * NCHUNK):
        ks = max(raw[c][seg][3] for c in range(NCORES))
        sizes = []
        for k in range(ks):
            mx = max(int((raw[c][seg][2] == k).sum()) for c in range(NCORES))
            sizes.append(-(-max(mx, 1) // 128) * 128)
        binsizes[seg] = sizes
        if ks:
            lo = slot
            spans = []
            for sz in sizes:
                spans.append((slot, slot + sz))
                slot += sz
            plan.append((seg // NCHUNK, seg % NCHUNK, lo, slot, spans))
    nslots = slot
    if nslots % 2048:
        pass  # slots are already multiples of 128; idx arrays use n/16 cols

    per_core = []
    rng = np.random.default_rng(1234)
    for c in range(NCORES):
        gi = np.zeros(nslots, np.int16)
        si = np.zeros(nslots, np.int16)
        pos = 0
        for (schunk, dchunk, lo, hi, spans) in plan:
            seg = schunk * NCHUNK + dchunk
            ss, dd, rank, _ = raw[c][seg]
            for k, (blo, bhi) in enumerate(spans):
                sz = bhi - blo
                m = rank == k
                bs = ss[m] - schunk * CH_REAL
                bd = dd[m] - dchunk * CH_REAL
                npad = sz - len(bs)
                assert npad >= 0
                if npad:
                    tp = CH_REAL + (np.arange(npad) % (CH_PAD - CH_REAL))
                    bs = np.concatenate([bs, np.zeros(npad, np.int64)])
                    bd = np.concatenate([bd, tp])
                gi[blo:bhi] = bs.astype(np.int16)
                si[blo:bhi] = bd.astype(np.int16)
        per_core.append(dict(gidx=gi, sidx=si))
    return plan, nslots, per_core


def _wrap16(a):
    """flat int16 index list (len % 16 == 0) -> [128, n/16] wrapped layout,
    replicated across the 8 GPSIMD core groups."""
    w = a.reshape(-1, 16).T.astype(np.int16)
    return np.ascontiguousarray(np.tile(w, (8, 1)))


def _build_program(plan, nslots, split=True):
    nc = bacc.Bacc("TRN2", target_bir_lowering=False, debug=False, num_devices=NCORES)
    AF = mybir.ActivationFunctionType

    t_ids16 = nc.dram_tensor("ids16", [128, NROW // 16], I16, kind="ExternalInput")
    t_batchf = nc.dram_tensor("batchf", [128, NCHUNK * FSLICE // 128], F32, kind="ExternalInput")
    t_gidx = nc.dram_tensor("gidx", [128, nslots // 16], I16, kind="ExternalInput")
    t_sidx = nc.dram_tensor("sidx", [128, nslots // 16], I16, kind="ExternalInput")
    t_fidx = nc.dram_tensor("fidx", [128, FSLICE // 16], I16, kind="ExternalInput")
    t_emb = nc.dram_tensor("emb", [NTYPES, EMB], F32, kind="ExternalInput")
    t_W1 = nc.dram_tensor("W1", [EMB, HID], F32, kind="ExternalInput")
    t_b1 = nc.dram_tensor("b1", [1, HID], F32, kind="ExternalInput")
    t_W2 = nc.dram_tensor("W2", [HID, HID], F32, kind="ExternalInput")
    t_b2 = nc.dram_tensor("b2", [1, HID], F32, kind="ExternalInput")
    t_Wc1 = nc.dram_tensor("Wc1", [HID, C1], F32, kind="ExternalInput")
    t_bc1 = nc.dram_tensor("bc1", [1, C1], F32, kind="ExternalInput")
    t_Wc2 = nc.dram_tensor("Wc2", [C1, 1], F32, kind="ExternalInput")
    t_bc2 = nc.dram_tensor("bc2", [1, 1], F32, kind="ExternalInput")
    t_iota = nc.dram_tensor("iota256", [128, G], F32, kind="ExternalInput")
    t_ident = nc.dram_tensor("ident128", [128, 128], F32, kind="ExternalInput")
    t_y = nc.dram_tensor("y", [G, 1], F32, kind="ExternalOutput")
    # zero-initialized by the runtime; reused as the t1 table after deg extraction
    t_deg = nc.dram_tensor("degtab", [NROW, EMB], F32, kind="ExternalOutput")

    t_u1 = nc.dram_tensor("u1tab", [NROW, EMB], F32)
    t_embw = nc.dram_tensor("embw", [256, EMB], F32)
    t_degc = nc.dram_tensor("degc", [NROW], F32)
    t_degr = nc.dram_tensor("degr", [NROW], F32, addr_space="Shared")
    acc1 = [nc.dram_tensor(f"acc1_{d}", [CH_PAD, EMB], F32) for d in range(NCHUNK)]
    acc2 = [nc.dram_tensor(f"acc2_{d}", [CH_PAD, EMB], F32) for d in range(NCHUNK)]
    acc1r = [nc.dram_tensor(f"acc1r_{d}", [CH_PAD, EMB], F32, addr_space="Shared")
             for d in range(NCHUNK)]
    acc2r = [nc.dram_tensor(f"acc2r_{d}", [CH_PAD, EMB], F32, addr_space="Shared")
             for d in range(NCHUNK)]
    t_pool = nc.dram_tensor("pooled", [G, 33], F32)
    t_poolr = nc.dram_tensor("pooledr", [G, 33], F32, addr_space="Shared")

    groups = [list(range(NCORES))]
    maxseg = max(hi - lo for (_, _, lo, hi, _) in plan)
    maxbin = max(bhi - blo for (*_, spans) in plan for (blo, bhi) in spans)

    with tile.TileContext(nc) as tc:
        with (
            tc.tile_pool(name="res", bufs=1) as res,
            tc.tile_pool(name="stage", bufs=2) as stage,
            tc.tile_pool(name="dense", bufs=3) as dense,
            tc.tile_pool(name="fine", bufs=3) as fine,
            tc.tile_pool(name="ps", bufs=2, space="PSUM") as psum,
            tc.tile_pool(name="poolacc", bufs=1, space="PSUM") as poolacc,
        ):
            # ---------- residents ----------
            ids16 = res.tile([128, NROW // 16], I16)
            nc.sync.dma_start(out=ids16[:], in_=t_ids16[:, :])
            gidx = res.tile([128, nslots // 16], I16)
            nc.sync.dma_start(out=gidx[:], in_=t_gidx[:, :])
            sidx = res.tile([128, nslots // 16], I16)
            nc.sync.dma_start(out=sidx[:], in_=t_sidx[:, :])
            fidx = res.tile([128, FSLICE // 16], I16)
            nc.sync.dma_start(out=fidx[:], in_=t_fidx[:, :])
            batchf = res.tile([128, NCHUNK * FSLICE // 128], F32)
            nc.sync.dma_start(out=batchf[:], in_=t_batchf[:, :])
            iota = res.tile([128, G], F32)
            nc.sync.dma_start(out=iota[:], in_=t_iota[:, :])
            ident = res.tile([128, 128], F32)
            nc.sync.dma_start(out=ident[:], in_=t_ident[:, :])
            onesP = res.tile([1, 128], F32)
            nc.vector.memset(onesP[:], 1.0)
            onestage = res.tile([128, maxbin // 128], F32)
            nc.vector.memset(onestage[:], 1.0)
            W1sb = res.tile([EMB, HID], F32)
            nc.sync.dma_start(out=W1sb[:], in_=t_W1[:, :])
            W2sb = res.tile([HID, HID], F32)
            nc.sync.dma_start(out=W2sb[:], in_=t_W2[:, :])
            Wc1sb = res.tile([HID, C1], F32)
            nc.sync.dma_start(out=Wc1sb[:], in_=t_Wc1[:, :])
            Wc2sb = res.tile([C1, 1], F32)
            nc.sync.dma_start(out=Wc2sb[:], in_=t_Wc2[:, :])

            def bcast_row(t_dram, w, nm):
                row = res.tile([1, w], F32, tag=f"row_{nm}")
                nc.sync.dma_start(out=row[:], in_=t_dram[:, :])
                p = psum.tile([128, w], F32, tag="pre")
                nc.tensor.matmul(out=p[:], lhsT=onesP[:], rhs=row[:], start=True, stop=True)
                out = res.tile([128, w], F32, tag=f"bc_{nm}")
                nc.vector.tensor_copy(out=out[:], in_=p[:])
                return out

            b1b = bcast_row(t_b1, HID, "b1")
            b2b = bcast_row(t_b2, HID, "b2")
            bc1b = bcast_row(t_bc1, C1, "bc1")
            bc2b = bcast_row(t_bc2, 1, "bc2")

            # ---------- embW1 = emb @ W1 ----------
            emb_lo = res.tile([128, EMB], F32)
            nc.sync.dma_start(out=emb_lo[:], in_=t_emb[0:128, :])
            ps1 = psum.tile([EMB, 128], F32, tag="pre")
            nc.tensor.transpose(out=ps1[:], in_=emb_lo[:], identity=ident[:])
            embT_lo = res.tile([EMB, 128], F32)
            nc.vector.tensor_copy(out=embT_lo[:], in_=ps1[:])
            emb_hi = res.tile([72, EMB], F32)
            nc.sync.dma_start(out=emb_hi[:], in_=t_emb[128:200, :])
            ps2 = psum.tile([EMB, 72], F32, tag="pre")
            nc.tensor.transpose(out=ps2[:], in_=emb_hi[:], identity=ident[0:72, 0:72])
            embT_hi = res.tile([EMB, 72], F32)
            nc.vector.tensor_copy(out=embT_hi[:], in_=ps2[:])
            ew_ps = psum.tile([128, HID], F32, tag="pre")
            nc.tensor.matmul(out=ew_ps[:], lhsT=embT_lo[:], rhs=W1sb[:], start=True, stop=True)
            ew_lo = res.tile([128, HID], F32)
            nc.vector.tensor_copy(out=ew_lo[:], in_=ew_ps[:])
            nc.sync.dma_start(out=t_embw[0:128, 0:HID], in_=ew_lo[:])
            ew_ps2 = psum.tile([72, HID], F32, tag="pre")
            nc.tensor.matmul(out=ew_ps2[:], lhsT=embT_hi[:], rhs=W1sb[:], start=True, stop=True)
            ew_hi = res.tile([72, HID], F32)
            nc.vector.tensor_copy(out=ew_hi[:], in_=ew_ps2[:])
            nc.sync.dma_start(out=t_embw[128:200, 0:HID], in_=ew_hi[:])

            # ---------- zero internal accumulators ----------
            zt = res.tile([128, 4096], F32)
            nc.vector.memset(zt[:], 0.0)
            for accs in (acc1, acc2):
                for a in accs:
                    av = a.ap().rearrange("(p q) e -> p (q e)", p=128)  # [128, 256*64]
                    for j in range(4):
                        nc.sync.dma_start(out=av[:, j * 4096:(j + 1) * 4096], in_=zt[:])

            # ---------- SWDGE serialization (descriptor-ring backpressure) ----------
            _sw = [None]

            def _chain(inst):
                if _sw[0] is not None:
                    add_dep_helper(inst.ins, _sw[0], reason="swdge chain")
                _sw[0] = inst.ins
                return inst

            SUB = 1024   # max indices per SWDGE op (ring capacity)

            # ---------- deg pass ----------
            for (schunk, dchunk, lo, hi, spans) in plan:
                dv = t_deg[dchunk * CH_PAD:(dchunk + 1) * CH_PAD, :]
                for (blo, bhi) in spans:
                    for q in range(blo, bhi, SUB):
                        qh = min(q + SUB, bhi)
                        nb = qh - q
                        _chain(nc.gpsimd.dma_scatter_add(
                            dv[:, 0:1], onestage[:, 0:nb // 128, None],
                            sidx[:, q // 16:qh // 16],
                            nb, nb, 1, elem_step=EMB, single_packet=False))

            # ---------- deg extraction + allreduce + c2 ----------
            for ch in range(NDENSE):
                dt_ = dense.tile([128, 16, EMB], F32, tag="dg_ld")
                nc.sync.dma_start(
                    out=dt_[:],
                    in_=t_deg[ch * 2048:(ch + 1) * 2048, :].rearrange("(p j) e -> p j e", p=128))
                dc = dense.tile([128, 16], F32, tag="dg_cp")
                nc.vector.tensor_copy(out=dc[:], in_=dt_[:, :, 0])
                nc.sync.dma_start(
                    out=t_degc[ch * 2048:(ch + 1) * 2048].rearrange("(p j) -> p j", p=128),
                    in_=dc[:])
            nc.gpsimd.collective_compute(
                "AllReduce", mybir.AluOpType.add, replica_groups=groups,
                ins=[t_degc.ap().opt()], outs=[t_degr.ap().opt()])

            c2 = res.tile([128, NDENSE, 16], F32)
            nc.sync.dma_start(
                out=c2[:], in_=t_degr.ap().rearrange("(g p j) -> p g j", p=128, j=16))
            nc.vector.tensor_scalar_max(out=c2[:], in0=c2[:], scalar1=1.0)
            nc.vector.reciprocal(out=c2[:], in_=c2[:])
            nc.scalar.activation(out=c2[:], in_=c2[:], func=AF.Sqrt)

            # ---------- t1 build (into degtab storage) ----------
            for ch in range(NDENSE):
                gt = dense.tile([128, 16, EMB], F32, tag="t1g")
                _chain(nc.gpsimd.dma_gather(
                    gt[:, 0:8, :], t_embw[:, :], ids16[:, ch * 128:ch * 128 + 64],
                    1024, 1024, EMB, single_packet=False))
                _chain(nc.gpsimd.dma_gather(
                    gt[:, 8:16, :], t_embw[:, :], ids16[:, ch * 128 + 64:(ch + 1) * 128],
                    1024, 1024, EMB, single_packet=False))
                cb = c2[:, ch, :, None].to_broadcast([128, 16, HID])
                t1c = dense.tile([128, 16, HID], F32, tag="t1c")
                nc.vector.tensor_mul(out=t1c[:], in0=gt[:, :, 0:HID], in1=cb)
                nc.sync.dma_start(
                    out=t_deg[ch * 2048:(ch + 1) * 2048, :]
                        .rearrange("(p j) e -> p j e", p=128)[:, :, 0:HID],
                    in_=t1c[:])

            # ---------- edge pass ----------
            def edge_pass(table, accs):
                for (schunk, dchunk, lo, hi, spans) in plan:
                    nseg = hi - lo
                    st = stage.tile([128, maxseg // 128, EMB], F32, tag="edgestage")
                    for q in range(0, nseg, SUB):
                        qh = min(q + SUB, nseg)
                        _chain(nc.gpsimd.dma_gather(
                            st[:, q // 128:qh // 128, :],
                            table[schunk * CH_PAD:(schunk + 1) * CH_PAD, :],
                            gidx[:, (lo + q) // 16:(lo + qh) // 16],
                            qh - q, qh - q, EMB, single_packet=False))
                    for (blo, bhi) in spans:
                        for q in range(blo, bhi, SUB):
                            qh = min(q + SUB, bhi)
                            nb = qh - q
                            _chain(nc.gpsimd.dma_scatter_add(
                                accs[dchunk][:, :],
                                st[:, (q - lo) // 128:(qh - lo) // 128, :],
                                sidx[:, q // 16:qh // 16],
                                nb, nb, EMB, single_packet=False))

            # ---------- L1 ----------
            edge_pass(t_deg, acc1)
            for d in range(NCHUNK):
                nc.gpsimd.collective_compute(
                    "AllReduce", mybir.AluOpType.add, replica_groups=groups,
                    ins=[acc1[d].ap().opt()], outs=[acc1r[d].ap().opt()])

            # ---------- interlayer: u1 = c * relu(c*acc1 + b1); col32 = 1 ----------
            b1v = b1b[:, None, :].to_broadcast([128, 16, HID])
            for ch in range(NDENSE):
                d = ch // DCH_PER
                off = (ch % DCH_PER) * 2048
                a = dense.tile([128, 16, EMB], F32, tag="il_ld")
                nc.sync.dma_start(
                    out=a[:],
                    in_=acc1r[d][off:off + 2048, :].rearrange("(p j) e -> p j e", p=128))
                cb = c2[:, ch, :, None].to_broadcast([128, 16, HID])
                y = dense.tile([128, 16, HID + 1], F32, tag="il_y")
                nc.vector.tensor_mul(out=y[:, :, 0:HID], in0=a[:, :, 0:HID], in1=cb)
                nc.vector.tensor_add(out=y[:, :, 0:HID], in0=y[:, :, 0:HID], in1=b1v)
                nc.scalar.activation(out=y[:, :, 0:HID], in_=y[:, :, 0:HID], func=AF.Relu)
                nc.vector.tensor_mul(out=y[:, :, 0:HID], in0=y[:, :, 0:HID], in1=cb)
                nc.vector.memset(y[:, :, HID:HID + 1], 1.0)
                nc.sync.dma_start(
                    out=t_u1[ch * 2048:(ch + 1) * 2048, :]
                        .rearrange("(p j) e -> p j e", p=128)[:, :, 0:HID + 1],
                    in_=y[:])

            # ---------- L2 ----------
            edge_pass(t_u1, acc2)
            for d in range(NCHUNK):
                nc.gpsimd.collective_compute(
                    "AllReduce", mybir.AluOpType.add, replica_groups=groups,
                    ins=[acc2[d].ap().opt()], outs=[acc2r[d].ap().opt()])

            # ---------- final: per-core slice via fidx gathers ----------
            pool_lo = poolacc.tile([128, 33], F32)
            pool_hi = poolacc.tile([128, 33], F32)
            NS = FSLICE // 128   # 32 subtiles per chunk
            first = True
            for d in range(NCHUNK):
                fs = fine.tile([128, NS, EMB], F32, tag="fstage")
                for q in range(0, FSLICE, SUB):
                    qh = q + SUB
                    _chain(nc.gpsimd.dma_gather(
                        fs[:, q // 128:qh // 128, :], acc2r[d][:, :],
                        fidx[:, q // 16:qh // 16],
                        SUB, SUB, EMB, single_packet=False))
                # c for these rows from deg in col 32
                cch = fine.tile([128, NS], F32, tag="fc")
                nc.vector.tensor_copy(out=cch[:], in_=fs[:, :, HID])
                nc.vector.tensor_scalar_max(out=cch[:], in0=cch[:], scalar1=1.0)
                nc.vector.reciprocal(out=cch[:], in_=cch[:])
                nc.scalar.activation(out=cch[:], in_=cch[:], func=AF.Sqrt)
                for s in range(NS):
                    y2T_ps = psum.tile([HID, 128], F32, tag="fin_t")
                    nc.tensor.transpose(out=y2T_ps[:], in_=fs[:, s, 0:HID], identity=ident[:])
                    y2T = fine.tile([HID, 128], F32, tag="fin_ts")
                    nc.vector.tensor_copy(out=y2T[:], in_=y2T_ps[:])
                    xw_ps = psum.tile([128, HID], F32, tag="fin_mm")
                    nc.tensor.matmul(out=xw_ps[:], lhsT=y2T[:], rhs=W2sb[:], start=True, stop=True)
                    h2e = fine.tile([128, 33], F32, tag="fin_h2")
                    nc.vector.tensor_scalar_mul(out=h2e[:, 0:HID], in0=xw_ps[:], scalar1=cch[:, s:s + 1])
                    nc.vector.tensor_add(out=h2e[:, 0:HID], in0=h2e[:, 0:HID], in1=b2b[:])
                    nc.scalar.activation(out=h2e[:, 0:HID], in_=h2e[:, 0:HID], func=AF.Relu)
                    nc.vector.memset(h2e[:, HID:HID + 1], 1.0)
                    oh = fine.tile([128, G], F32, tag="fin_oh")
                    nc.vector.tensor_tensor(
                        out=oh[:], in0=batchf[:, d * NS + s, None].to_broadcast([128, G]),
                        in1=iota[:], op=mybir.AluOpType.is_equal)
                    nc.tensor.matmul(out=pool_lo[:], lhsT=oh[:, 0:128], rhs=h2e[:],
                                     start=first, stop=(d == NCHUNK - 1 and s == NS - 1))
                    nc.tensor.matmul(out=pool_hi[:], lhsT=oh[:, 128:G], rhs=h2e[:],
                                     start=first, stop=(d == NCHUNK - 1 and s == NS - 1))
                    first = False

            pl = res.tile([128, 33], F32, tag="pl")
            ph = res.tile([128, 33], F32, tag="ph")
            nc.vector.tensor_copy(out=pl[:], in_=pool_lo[:])
            nc.vector.tensor_copy(out=ph[:], in_=pool_hi[:])
            nc.sync.dma_start(out=t_pool[0:128, :], in_=pl[:])
            nc.sync.dma_start(out=t_pool[128:G, :], in_=ph[:])
            nc.gpsimd.collective_compute(
                "AllReduce", mybir.AluOpType.add, replica_groups=groups,
                ins=[t_pool.ap().opt()], outs=[t_poolr.ap().opt()])

            # ---------- mean + MLP head (every core computes the same y) ----------
            for half in range(2):
                pr = res.tile([128, 33], F32, tag=f"pr{half}")
                nc.sync.dma_start(out=pr[:], in_=t_poolr[half * 128:(half + 1) * 128, :])
                cnt = res.tile([128, 1], F32, tag=f"cnt{half}")
                nc.vector.tensor_scalar_max(out=cnt[:], in0=pr[:, 32:33], scalar1=1.0)
                nc.vector.reciprocal(out=cnt[:], in_=cnt[:])
                mean = res.tile([128, HID], F32, tag=f"mean{half}")
                nc.vector.tensor_scalar_mul(out=mean[:], in0=pr[:, 0:HID], scalar1=cnt[:])
                mT_ps = psum.tile([HID, 128], F32, tag="pre")
                nc.tensor.transpose(out=mT_ps[:], in_=mean[:], identity=ident[:])
                mT = res.tile([HID, 128], F32, tag=f"mT{half}")
                nc.vector.tensor_copy(out=mT[:], in_=mT_ps[:])
                hc_ps = psum.tile([128, C1], F32, tag="pre")
                nc.tensor.matmul(out=hc_ps[:], lhsT=mT[:], rhs=Wc1sb[:], start=True, stop=True)
                hc = res.tile([128, C1], F32, tag=f"hc{half}")
                nc.vector.tensor_add(out=hc[:], in0=hc_ps[:], in1=bc1b[:])
                nc.scalar.activation(out=hc[:], in_=hc[:], func=AF.Relu)
                hT_ps = psum.tile([C1, 128], F32, tag="pre")
                nc.tensor.transpose(out=hT_ps[:], in_=hc[:], identity=ident[:])
                hT = res.tile([C1, 128], F32, tag=f"hT{half}")
                nc.vector.tensor_copy(out=hT[:], in_=hT_ps[:])
                o_ps = psum.tile([128, 1], F32, tag="pre")
                nc.tensor.matmul(out=o_ps[:], lhsT=hT[:], rhs=Wc2sb[:], start=True, stop=True)
                ob = res.tile([128, 1], F32, tag=f"ob{half}")
                nc.vector.tensor_add(out=ob[:], in0=o_ps[:], in1=bc2b[:])
                nc.scalar.activation(out=ob[:], in_=ob[:], func=AF.Sigmoid)
                nc.sync.dma_start(out=t_y[half * 128:(half + 1) * 128, :], in_=ob[:])

    nc.compile()
    if split:
        _split_sync_waits(nc)
    return nc


_PROG_CACHE = {}


def kernel(**inputs):
    x = np.asarray(inputs["x"]).astype(np.int64).reshape(-1)
    ei = np.asarray(inputs["edge_index"]).astype(np.int64)
    batch = np.asarray(inputs["batch"]).astype(np.int64).reshape(-1)
    emb = np.asarray(inputs["emb"], np.float32)
    W1 = np.asarray(inputs["W1"], np.float32)
    b1 = np.asarray(inputs["b1"], np.float32).reshape(1, -1)
    W2 = np.asarray(inputs["W2"], np.float32)
    b2 = np.asarray(inputs["b2"], np.float32).reshape(1, -1)
    Wc1 = np.asarray(inputs["Wc1"], np.float32)
    bc1 = np.asarray(inputs["bc1"], np.float32).reshape(1, -1)
    Wc2 = np.asarray(inputs["Wc2"], np.float32)
    bc2 = np.asarray(inputs["bc2"], np.float32).reshape(1, -1)

    plan, nslots, per_core = _shard_edges(ei[0], ei[1])

    key = (nslots, tuple((s, d, lo, hi, tuple(sp)) for (s, d, lo, hi, sp) in plan))
    if key not in _PROG_CACHE:
        _PROG_CACHE[key] = _build_program(plan, nslots)
    nc = _PROG_CACHE[key]

    # node-id table in padded row space (row = (n//CH_REAL)*CH_PAD + n%CH_REAL)
    ids_pad = np.zeros(NROW, np.int64)
    rows = (np.arange(N) // CH_REAL) * CH_PAD + (np.arange(N) % CH_REAL)
    ids_pad[rows] = x
    ids16 = _wrap16(ids_pad.astype(np.int16))

    # batch values for each core's final-phase rows: core c, chunk d,
    # subtile s, partition p -> chunk-local row FSLICE*c + s*128 + p
    iota256 = np.tile(np.arange(G, dtype=np.float32), (128, 1))
    ident128 = np.eye(128, dtype=np.float32)

    loc = np.arange(FSLICE)
    in_maps = []
    for c in range(NCORES):
        fl = FSLICE * c + loc          # chunk-local rows this core handles
        fidx = _wrap16(fl.astype(np.int16))
        bvals = np.full((NCHUNK, FSLICE), -1.0, np.float32)
        for d in range(NCHUNK):
            gl = fl.copy()
            real = gl < CH_REAL
            n_global = d * CH_REAL + gl
            ok = real & (n_global < N)
            bvals[d, ok] = batch[n_global[ok]]
        # [128, NCHUNK*FSLICE/128] with col d*NS + s at partition p = row s*128+p
        barr = bvals.reshape(NCHUNK, FSLICE // 128, 128).transpose(2, 0, 1).reshape(
            128, NCHUNK * (FSLICE // 128))
        in_maps.append(dict(
            ids16=ids16,
            batchf=np.ascontiguousarray(barr),
            gidx=_wrap16(per_core[c]["gidx"]),
            sidx=_wrap16(per_core[c]["sidx"]),
            fidx=fidx,
            emb=emb, W1=W1, b1=b1, W2=W2, b2=b2,
            Wc1=Wc1, bc1=bc1, Wc2=Wc2, bc2=bc2,
            iota256=iota256, ident128=ident128,
        ))

    res = run_bass_kernel_spmd(nc, in_maps, core_ids=list(range(NCORES)))
    return res.results[0]["y"].astype(np.float32)



# revision 19
# speedup vs baseline: 6257.5011x; 6257.5011x over previous
"""Trainium2 Bass kernel for a 2-layer GCN + global mean pool + MLP head.

Distribution (8 NeuronCores): edge-parallel. Edges (plus one self-loop per
node) are sharded across cores as part of input distribution; each core
gathers node-table rows by src (dma_gather) and scatter-adds them by dst
(dma_scatter_add with SDMA CCE f32 add) into per-core partial accumulators;
node-boundary partial sums are combined with AllReduce. Small parameters are
replicated.

Math: with c = rsqrt(deg) (deg counts in-edges incl. the self loop), each
GCN layer is  h' = relu(c * (sum_{u->v} t[u]) + b)  with  t = c * (h @ W).
The layer-2 weight multiply commutes with the edge sum, so the second edge
pass scatters u1 = c * h1 rows and W2 is applied after the reduce. Column 32
of the u1 rows carries the constant 1, so acc2[:,32] reproduces deg and the
final phase is self-contained per gathered row.

Race-freedom: duplicate scatter destinations within one dma_scatter_add and
across concurrently-running ones are not accumulated correctly by the DMA
engines, so the host deals each (src-chunk, dst-chunk) edge segment into
bins with unique dst (rank-within-dst dealing), and all scatter instructions
that target the same dst-chunk accumulator are chained with explicit deps.
"""

import numpy as np

import concourse.bacc as bacc
import concourse.mybir as mybir
import concourse.tile as tile
from concourse.bass_utils import run_bass_kernel_spmd
from bass_rust import add_dep_helper

# ---- problem geometry (hardcoded per task contract) ----
N = 100000
E = 1000000
G = 256
NTYPES = 200
EMB = 64            # embedding dim; also the 256B table row width (64 f32)
HID = 32
C1 = 16
NCORES = 8

CH_REAL = 25600     # real node rows per chunk (int16-addressable)
CH_PAD = 32768      # chunk stride (16 * 2048)
NCHUNK = 4
NROW = NCHUNK * CH_PAD            # 131072 padded rows
NDENSE = NROW // 2048             # 64 dense chunks
DCH_PER = CH_PAD // 2048          # 16 dense chunks per node chunk
FSLICE = CH_PAD // NCORES         # 4096 rows per core per chunk (final phase)
F32 = mybir.dt.float32
I16 = mybir.dt.int16
I32 = mybir.dt.int32

MAX_WAITS = 1


def _split_sync_waits(nc):
    """walrus TPB codegen encodes at most one sync-wait per instruction;
    split longer wait lists into preceding same-engine nops."""
    n = 0
    for f in nc.m.functions:
        for blk in f.blocks:
            il = blk.instructions
            i = 0
            while i < len(il):
                ins = il[i]
                si = ins.sync_info
                if si is not None and si.on_wait and len(si.on_wait) > MAX_WAITS:
                    w = list(si.on_wait)
                    si.on_wait = w[-MAX_WAITS:]
                    ex = w[:-MAX_WAITS]
                    nops = []
                    for k in range(0, len(ex), MAX_WAITS):
                        p = mybir.InstNoOp(name=f"Wsplit-{n}-{k}")
                        p.engine = ins.engine
                        p.sync_info = mybir.SyncInfo(on_wait=ex[k:k + MAX_WAITS], on_update=[])
                        nops.append(p)
                    for j, p in enumerate(nops):
                        il.insert(i + j, p)
                    i += len(nops)
                    n += 1
                i += 1
    return n


def _shard_edges(src, dst):
    """Shard edges + self loops across cores; group per (src_chunk,
    dst_chunk); deal into unique-dst bins; pad bins to a structure common to
    all cores (the SPMD program is shared). Returns (plan, per_core) where
    plan = [(s, d, seg_lo, seg_hi, [(bin_lo, bin_hi), ...])] in slot units
    and per_core = list of dicts with int16 gidx/sidx flat slot arrays.
    """
    selfn = np.arange(N, dtype=np.int64)
    e_core = np.arange(E) % NCORES
    s_core = selfn % NCORES
    raw = []   # raw[c][seg] = (ss, dd, rank, ks)
    for c in range(NCORES):
        s = np.concatenate([src[e_core == c], selfn[s_core == c]])
        d = np.concatenate([dst[e_core == c], selfn[s_core == c]])
        seg_key = (s // CH_REAL) * NCHUNK + (d // CH_REAL)
        segs = {}
        for seg in range(NCHUNK * NCHUNK):
            m = seg_key == seg
            ss, dd = s[m], d[m]
            if len(dd):
                do = np.argsort(dd, kind="stable")
                ss, dd = ss[do], dd[do]
                grp = np.flatnonzero(np.r_[True, dd[1:] != dd[:-1]])
                rank = np.arange(len(dd)) - np.repeat(grp, np.diff(np.r_[grp, len(dd)]))
                ks = int(rank.max()) + 1
            else:
                rank, ks = np.zeros(0, np.int64), 0
            segs[seg] = (ss, dd, rank, ks)
        raw.append(segs)

    # common bin sizes (padded to 128 slots)
    plan = []
    slot = 0
    binsizes = {}
    for seg in range(NCHUNK * NCHUNK):
        ks = max(raw[c][seg][3] for c in range(NCORES))
        sizes = []
        for k in range(ks):
            mx = max(int((raw[c][seg][2] == k).sum()) for c in range(NCORES))
            sizes.append(-(-max(mx, 1) // 128) * 128)
        binsizes[seg] = sizes
        if ks:
            lo = slot
            spans = []
            for sz in sizes:
                spans.append((slot, slot + sz))
                slot += sz
            plan.append((seg // NCHUNK, seg % NCHUNK, lo, slot, spans))
    nslots = slot
    if nslots % 2048:
        pass  # slots are already multiples of 128; idx arrays use n/16 cols

    per_core = []
    rng = np.random.default_rng(1234)
    for c in range(NCORES):
        gi = np.zeros(nslots, np.int16)
        si = np.zeros(nslots, np.int16)
        pos = 0
        for (schunk, dchunk, lo, hi, spans) in plan:
            seg = schunk * NCHUNK + dchunk
            ss, dd, rank, _ = raw[c][seg]
            for k, (blo, bhi) in enumerate(spans):
                sz = bhi - blo
                m = rank == k
                bs = ss[m] - schunk * CH_REAL
                bd = dd[m] - dchunk * CH_REAL
                npad = sz - len(bs)
                assert npad >= 0
                if npad:
                    tp = CH_REAL + (np.arange(npad) % (CH_PAD - CH_REAL))
                    bs = np.concatenate([bs, np.zeros(npad, np.int64)])
                    bd = np.concatenate([bd, tp])
                gi[blo:bhi] = bs.astype(np.int16)
                si[blo:bhi] = bd.astype(np.int16)
        per_core.append(dict(gidx=gi, sidx=si))
    return plan, nslots, per_core


def _wrap16(a):
    """flat int16 index list (len % 16 == 0) -> [128, n/16] wrapped layout,
    replicated across the 8 GPSIMD core groups."""
    w = a.reshape(-1, 16).T.astype(np.int16)
    return np.ascontiguousarray(np.tile(w, (8, 1)))


def _build_program(plan, nslots, split=True):
    nc = bacc.Bacc("TRN2", target_bir_lowering=False, debug=False, num_devices=NCORES,
                   dynamic_dma_scratch_size=32768, num_swdge_queues=2)
    AF = mybir.ActivationFunctionType

    t_ids16 = nc.dram_tensor("ids16", [128, NROW // 16], I16, kind="ExternalInput")
    t_batchf = nc.dram_tensor("batchf", [128, NCHUNK * FSLICE // 128], F32, kind="ExternalInput")
    t_gidx = nc.dram_tensor("gidx", [128, nslots // 16], I16, kind="ExternalInput")
    t_sidx = nc.dram_tensor("sidx", [128, nslots // 16], I16, kind="ExternalInput")
    t_fidx = nc.dram_tensor("fidx", [128, FSLICE // 16], I16, kind="ExternalInput")
    t_emb = nc.dram_tensor("emb", [NTYPES, EMB], F32, kind="ExternalInput")
    t_W1 = nc.dram_tensor("W1", [EMB, HID], F32, kind="ExternalInput")
    t_b1 = nc.dram_tensor("b1", [1, HID], F32, kind="ExternalInput")
    t_W2 = nc.dram_tensor("W2", [HID, HID], F32, kind="ExternalInput")
    t_b2 = nc.dram_tensor("b2", [1, HID], F32, kind="ExternalInput")
    t_Wc1 = nc.dram_tensor("Wc1", [HID, C1], F32, kind="ExternalInput")
    t_bc1 = nc.dram_tensor("bc1", [1, C1], F32, kind="ExternalInput")
    t_Wc2 = nc.dram_tensor("Wc2", [C1, 1], F32, kind="ExternalInput")
    t_bc2 = nc.dram_tensor("bc2", [1, 1], F32, kind="ExternalInput")
    t_iota = nc.dram_tensor("iota256", [128, G], F32, kind="ExternalInput")
    t_ident = nc.dram_tensor("ident128", [128, 128], F32, kind="ExternalInput")
    t_degf = nc.dram_tensor("degf", [NROW], F32, kind="ExternalInput")
    t_y = nc.dram_tensor("y", [G, 1], F32, kind="ExternalOutput")
    # t1 table: all rows' cols 0:HID are written by the t1 build before any
    # gather reads them; cols HID:EMB are never consumed downstream.
    t_deg = nc.dram_tensor("degtab", [NROW, EMB], F32)

    t_u1 = nc.dram_tensor("u1tab", [NROW, EMB], F32)
    t_embw = nc.dram_tensor("embw", [256, EMB], F32)
    # single fused accumulator per layer: one big AllReduce reaches the
    # high-bandwidth collective tier instead of 4 small ones
    t_acc1 = nc.dram_tensor("acc1", [NROW, EMB], F32)
    t_acc2 = nc.dram_tensor("acc2", [NROW, EMB], F32)
    t_acc1r = nc.dram_tensor("acc1r", [NROW, EMB], F32, addr_space="Shared")
    t_acc2r = nc.dram_tensor("acc2r", [NROW, EMB], F32, addr_space="Shared")
    t_pool = nc.dram_tensor("pooled", [G, 33], F32)
    t_poolr = nc.dram_tensor("pooledr", [G, 33], F32, addr_space="Shared")

    groups = [list(range(NCORES))]
    maxseg = max(hi - lo for (_, _, lo, hi, _) in plan)
    maxbin = max(bhi - blo for (*_, spans) in plan for (blo, bhi) in spans)

    with tile.TileContext(nc) as tc:
        with (
            tc.tile_pool(name="res", bufs=1) as res,
            tc.tile_pool(name="stage", bufs=2) as stage,
            tc.tile_pool(name="dense", bufs=3) as dense,
            tc.tile_pool(name="fine", bufs=2) as fine,
            tc.tile_pool(name="ps", bufs=2, space="PSUM") as psum,
            tc.tile_pool(name="poolacc", bufs=1, space="PSUM") as poolacc,
        ):
            # ---------- residents ----------
            ids16 = res.tile([128, NROW // 16], I16)
            nc.sync.dma_start(out=ids16[:], in_=t_ids16[:, :])
            gidx = res.tile([128, nslots // 16], I16)
            nc.sync.dma_start(out=gidx[:], in_=t_gidx[:, :])
            sidx = res.tile([128, nslots // 16], I16)
            nc.sync.dma_start(out=sidx[:], in_=t_sidx[:, :])
            fidx = res.tile([128, FSLICE // 16], I16)
            nc.sync.dma_start(out=fidx[:], in_=t_fidx[:, :])
            batchf = res.tile([128, NCHUNK * FSLICE // 128], F32)
            nc.sync.dma_start(out=batchf[:], in_=t_batchf[:, :])
            iota = res.tile([128, G], F32)
            nc.sync.dma_start(out=iota[:], in_=t_iota[:, :])
            ident = res.tile([128, 128], F32)
            nc.sync.dma_start(out=ident[:], in_=t_ident[:, :])
            onesP = res.tile([1, 128], F32)
            nc.vector.memset(onesP[:], 1.0)
            W1sb = res.tile([EMB, HID], F32)
            nc.sync.dma_start(out=W1sb[:], in_=t_W1[:, :])
            W2sb = res.tile([HID, HID], F32)
            nc.sync.dma_start(out=W2sb[:], in_=t_W2[:, :])
            Wc1sb = res.tile([HID, C1], F32)
            nc.sync.dma_start(out=Wc1sb[:], in_=t_Wc1[:, :])
            Wc2sb = res.tile([C1, 1], F32)
            nc.sync.dma_start(out=Wc2sb[:], in_=t_Wc2[:, :])

            def bcast_row(t_dram, w, nm):
                row = res.tile([1, w], F32, tag=f"row_{nm}")
                nc.sync.dma_start(out=row[:], in_=t_dram[:, :])
                p = psum.tile([128, w], F32, tag="pre")
                nc.tensor.matmul(out=p[:], lhsT=onesP[:], rhs=row[:], start=True, stop=True)
                out = res.tile([128, w], F32, tag=f"bc_{nm}")
                nc.vector.tensor_copy(out=out[:], in_=p[:])
                return out

            b1b = bcast_row(t_b1, HID, "b1")
            b2b = bcast_row(t_b2, HID, "b2")
            bc1b = bcast_row(t_bc1, C1, "bc1")
            bc2b = bcast_row(t_bc2, 1, "bc2")

            # ---------- embW1 = emb @ W1 ----------
            emb_lo = res.tile([128, EMB], F32)
            nc.sync.dma_start(out=emb_lo[:], in_=t_emb[0:128, :])
            ps1 = psum.tile([EMB, 128], F32, tag="pre")
            nc.tensor.transpose(out=ps1[:], in_=emb_lo[:], identity=ident[:])
            embT_lo = res.tile([EMB, 128], F32)
            nc.vector.tensor_copy(out=embT_lo[:], in_=ps1[:])
            emb_hi = res.tile([72, EMB], F32)
            nc.sync.dma_start(out=emb_hi[:], in_=t_emb[128:200, :])
            ps2 = psum.tile([EMB, 72], F32, tag="pre")
            nc.tensor.transpose(out=ps2[:], in_=emb_hi[:], identity=ident[0:72, 0:72])
            embT_hi = res.tile([EMB, 72], F32)
            nc.vector.tensor_copy(out=embT_hi[:], in_=ps2[:])
            ew_ps = psum.tile([128, HID], F32, tag="pre")
            nc.tensor.matmul(out=ew_ps[:], lhsT=embT_lo[:], rhs=W1sb[:], start=True, stop=True)
            ew_lo = res.tile([128, HID], F32)
            nc.vector.tensor_copy(out=ew_lo[:], in_=ew_ps[:])
            nc.sync.dma_start(out=t_embw[0:128, 0:HID], in_=ew_lo[:])
            ew_ps2 = psum.tile([72, HID], F32, tag="pre")
            nc.tensor.matmul(out=ew_ps2[:], lhsT=embT_hi[:], rhs=W1sb[:], start=True, stop=True)
            ew_hi = res.tile([72, HID], F32)
            nc.vector.tensor_copy(out=ew_hi[:], in_=ew_ps2[:])
            nc.sync.dma_start(out=t_embw[128:200, 0:HID], in_=ew_hi[:])

            # ---------- zero internal accumulators ----------
            zt = res.tile([128, 4096], F32)
            nc.vector.memset(zt[:], 0.0)
            for t_acc in (t_acc1, t_acc2):
                av = t_acc.ap().rearrange("(p q) e -> p (q e)", p=128)  # [128, 1024*64]
                for j in range(16):
                    nc.sync.dma_start(out=av[:, j * 4096:(j + 1) * 4096], in_=zt[:])

            # ---------- SWDGE serialization (descriptor-ring backpressure) ----------
            # two queues with independent rings: gathers on q0, scatters on q1,
            # each chained within its queue so DGE of one overlaps DMA of the other
            _sw = [None, None]

            def _chain(inst, q=0):
                if _sw[q] is not None:
                    add_dep_helper(inst.ins, _sw[q], reason="swdge chain")
                _sw[q] = inst.ins
                return inst

            SUB = 2048   # max indices per SWDGE op (per-queue ring capacity)

            # ---------- c2 = rsqrt(max(deg,1)) from host-computed degrees ----------
            c2 = res.tile([128, NDENSE, 16], F32)
            nc.sync.dma_start(
                out=c2[:], in_=t_degf.ap().rearrange("(g p j) -> p g j", p=128, j=16))
            nc.vector.tensor_scalar_max(out=c2[:], in0=c2[:], scalar1=1.0)
            nc.vector.reciprocal(out=c2[:], in_=c2[:])
            nc.scalar.activation(out=c2[:], in_=c2[:], func=AF.Sqrt)

            # ---------- t1 build (into degtab storage) ----------
            for ch in range(NDENSE):
                gt = dense.tile([128, 16, EMB], F32, tag="t1g")
                _chain(nc.gpsimd.dma_gather(
                    gt[:, 0:8, :], t_embw[:, :], ids16[:, ch * 128:ch * 128 + 64],
                    1024, 1024, EMB, single_packet=False))
                _chain(nc.gpsimd.dma_gather(
                    gt[:, 8:16, :], t_embw[:, :], ids16[:, ch * 128 + 64:(ch + 1) * 128],
                    1024, 1024, EMB, single_packet=False))
                cb = c2[:, ch, :, None].to_broadcast([128, 16, HID])
                t1c = dense.tile([128, 16, HID], F32, tag="t1c")
                nc.vector.tensor_mul(out=t1c[:], in0=gt[:, :, 0:HID], in1=cb)
                nc.sync.dma_start(
                    out=t_deg[ch * 2048:(ch + 1) * 2048, :]
                        .rearrange("(p j) e -> p j e", p=128)[:, :, 0:HID],
                    in_=t1c[:])

            # ---------- edge pass ----------
            def edge_pass(table, t_acc):
                for (schunk, dchunk, lo, hi, spans) in plan:
                    nseg = hi - lo
                    st = stage.tile([128, maxseg // 128, EMB], F32, tag="edgestage")
                    for q in range(0, nseg, SUB):
                        qh = min(q + SUB, nseg)
                        _chain(nc.gpsimd.dma_gather(
                            st[:, q // 128:qh // 128, :],
                            table[schunk * CH_PAD:(schunk + 1) * CH_PAD, :],
                            gidx[:, (lo + q) // 16:(lo + qh) // 16],
                            qh - q, qh - q, EMB, single_packet=False))
                    for (blo, bhi) in spans:
                        for q in range(blo, bhi, SUB):
                            qh = min(q + SUB, bhi)
                            nb = qh - q
                            _chain(nc.gpsimd.dma_scatter_add(
                                t_acc[dchunk * CH_PAD:(dchunk + 1) * CH_PAD, :],
                                st[:, (q - lo) // 128:(qh - lo) // 128, :],
                                sidx[:, q // 16:qh // 16],
                                nb, nb, EMB, single_packet=False, queue_num=1), q=1)

            # ---------- L1 ----------
            edge_pass(t_deg, t_acc1)
            nc.gpsimd.collective_compute(
                "AllReduce", mybir.AluOpType.add, replica_groups=groups,
                ins=[t_acc1.ap().opt()], outs=[t_acc1r.ap().opt()])

            # ---------- interlayer: u1 = c * relu(c*acc1 + b1); col32 = 1 ----------
            b1v = b1b[:, None, :].to_broadcast([128, 16, HID])
            for ch in range(NDENSE):
                d = ch // DCH_PER
                off = (ch % DCH_PER) * 2048
                a = dense.tile([128, 16, EMB], F32, tag="il_ld")
                nc.sync.dma_start(
                    out=a[:],
                    in_=t_acc1r[ch * 2048:(ch + 1) * 2048, :]
                        .rearrange("(p j) e -> p j e", p=128))
                cb = c2[:, ch, :, None].to_broadcast([128, 16, HID])
                y = dense.tile([128, 16, HID + 1], F32, tag="il_y")
                nc.vector.tensor_mul(out=y[:, :, 0:HID], in0=a[:, :, 0:HID], in1=cb)
                nc.vector.tensor_add(out=y[:, :, 0:HID], in0=y[:, :, 0:HID], in1=b1v)
                nc.scalar.activation(out=y[:, :, 0:HID], in_=y[:, :, 0:HID], func=AF.Relu)
                nc.vector.tensor_mul(out=y[:, :, 0:HID], in0=y[:, :, 0:HID], in1=cb)
                nc.vector.memset(y[:, :, HID:HID + 1], 1.0)
                nc.sync.dma_start(
                    out=t_u1[ch * 2048:(ch + 1) * 2048, :]
                        .rearrange("(p j) e -> p j e", p=128)[:, :, 0:HID + 1],
                    in_=y[:])

            # ---------- L2 ----------
            edge_pass(t_u1, t_acc2)
            nc.gpsimd.collective_compute(
                "AllReduce", mybir.AluOpType.add, replica_groups=groups,
                ins=[t_acc2.ap().opt()], outs=[t_acc2r.ap().opt()])

            # ---------- final: per-core slice via fidx gathers ----------
            pool_lo = poolacc.tile([128, 33], F32)
            pool_hi = poolacc.tile([128, 33], F32)
            NS = FSLICE // 128   # 32 subtiles per chunk
            first = True
            for d in range(NCHUNK):
                fs = fine.tile([128, NS, EMB], F32, tag="fstage")
                for q in range(0, FSLICE, SUB):
                    qh = min(q + SUB, FSLICE)
                    _chain(nc.gpsimd.dma_gather(
                        fs[:, q // 128:qh // 128, :],
                        t_acc2r[d * CH_PAD:(d + 1) * CH_PAD, :],
                        fidx[:, q // 16:qh // 16],
                        qh - q, qh - q, EMB, single_packet=False))
                # c for these rows from deg in col 32
                cch = fine.tile([128, NS], F32, tag="fc")
                nc.vector.tensor_copy(out=cch[:], in_=fs[:, :, HID])
                nc.vector.tensor_scalar_max(out=cch[:], in0=cch[:], scalar1=1.0)
                nc.vector.reciprocal(out=cch[:], in_=cch[:])
                nc.scalar.activation(out=cch[:], in_=cch[:], func=AF.Sqrt)
                for s in range(NS):
                    y2T_ps = psum.tile([HID, 128], F32, tag="fin_t")
                    nc.tensor.transpose(out=y2T_ps[:], in_=fs[:, s, 0:HID], identity=ident[:])
                    y2T = fine.tile([HID, 128], F32, tag="fin_ts")
                    nc.vector.tensor_copy(out=y2T[:], in_=y2T_ps[:])
                    xw_ps = psum.tile([128, HID], F32, tag="fin_mm")
                    nc.tensor.matmul(out=xw_ps[:], lhsT=y2T[:], rhs=W2sb[:], start=True, stop=True)
                    h2e = fine.tile([128, 33], F32, tag="fin_h2")
                    nc.vector.tensor_scalar_mul(out=h2e[:, 0:HID], in0=xw_ps[:], scalar1=cch[:, s:s + 1])
                    nc.vector.tensor_add(out=h2e[:, 0:HID], in0=h2e[:, 0:HID], in1=b2b[:])
                    nc.scalar.activation(out=h2e[:, 0:HID], in_=h2e[:, 0:HID], func=AF.Relu)
                    nc.vector.memset(h2e[:, HID:HID + 1], 1.0)
                    oh = fine.tile([128, G], F32, tag="fin_oh")
                    nc.vector.tensor_tensor(
                        out=oh[:], in0=batchf[:, d * NS + s, None].to_broadcast([128, G]),
                        in1=iota[:], op=mybir.AluOpType.is_equal)
                    nc.tensor.matmul(out=pool_lo[:], lhsT=oh[:, 0:128], rhs=h2e[:],
                                     start=first, stop=(d == NCHUNK - 1 and s == NS - 1))
                    nc.tensor.matmul(out=pool_hi[:], lhsT=oh[:, 128:G], rhs=h2e[:],
                                     start=first, stop=(d == NCHUNK - 1 and s == NS - 1))
                    first = False

            pl = res.tile([128, 33], F32, tag="pl")
            ph = res.tile([128, 33], F32, tag="ph")
            nc.vector.tensor_copy(out=pl[:], in_=pool_lo[:])
            nc.vector.tensor_copy(out=ph[:], in_=pool_hi[:])
            nc.sync.dma_start(out=t_pool[0:128, :], in_=pl[:])
            nc.sync.dma_start(out=t_pool[128:G, :], in_=ph[:])
            nc.gpsimd.collective_compute(
                "AllReduce", mybir.AluOpType.add, replica_groups=groups,
                ins=[t_pool.ap().opt()], outs=[t_poolr.ap().opt()])

            # ---------- mean + MLP head (every core computes the same y) ----------
            for half in range(2):
                pr = res.tile([128, 33], F32, tag=f"pr{half}")
                nc.sync.dma_start(out=pr[:], in_=t_poolr[half * 128:(half + 1) * 128, :])
                cnt = res.tile([128, 1], F32, tag=f"cnt{half}")
                nc.vector.tensor_scalar_max(out=cnt[:], in0=pr[:, 32:33], scalar1=1.0)
                nc.vector.reciprocal(out=cnt[:], in_=cnt[:])
                mean = res.tile([128, HID], F32, tag=f"mean{half}")
                nc.vector.tensor_scalar_mul(out=mean[:], in0=pr[:, 0:HID], scalar1=cnt[:])
                mT_ps = psum.tile([HID, 128], F32, tag="pre")
                nc.tensor.transpose(out=mT_ps[:], in_=mean[:], identity=ident[:])
                mT = res.tile([HID, 128], F32, tag=f"mT{half}")
                nc.vector.tensor_copy(out=mT[:], in_=mT_ps[:])
                hc_ps = psum.tile([128, C1], F32, tag="pre")
                nc.tensor.matmul(out=hc_ps[:], lhsT=mT[:], rhs=Wc1sb[:], start=True, stop=True)
                hc = res.tile([128, C1], F32, tag=f"hc{half}")
                nc.vector.tensor_add(out=hc[:], in0=hc_ps[:], in1=bc1b[:])
                nc.scalar.activation(out=hc[:], in_=hc[:], func=AF.Relu)
                hT_ps = psum.tile([C1, 128], F32, tag="pre")
                nc.tensor.transpose(out=hT_ps[:], in_=hc[:], identity=ident[:])
                hT = res.tile([C1, 128], F32, tag=f"hT{half}")
                nc.vector.tensor_copy(out=hT[:], in_=hT_ps[:])
                o_ps = psum.tile([128, 1], F32, tag="pre")
                nc.tensor.matmul(out=o_ps[:], lhsT=hT[:], rhs=Wc2sb[:], start=True, stop=True)
                ob = res.tile([128, 1], F32, tag=f"ob{half}")
                nc.vector.tensor_add(out=ob[:], in0=o_ps[:], in1=bc2b[:])
                nc.scalar.activation(out=ob[:], in_=ob[:], func=AF.Sigmoid)
                nc.sync.dma_start(out=t_y[half * 128:(half + 1) * 128, :], in_=ob[:])

    nc.compile()
    if split:
        _split_sync_waits(nc)
    return nc


_PROG_CACHE = {}
_RUN_CACHE = {}


def _make_runner(nc):
    """Cached PJRT runner: trace/jit the shard_map body once per program and
    reuse it across kernel() calls (run_bass_kernel_spmd re-jits every call)."""
    import jax
    from jax.experimental.shard_map import shard_map
    from jax.sharding import Mesh, PartitionSpec
    from concourse import bass2jax

    bass2jax.install_neuronx_cc_hook()
    assert nc.dbg_addr is None
    partition_name = nc.partition_id_tensor.name if nc.partition_id_tensor else None

    in_names, out_names, out_avals, zero_specs = [], [], [], []
    for alloc in nc.m.functions[0].allocations:
        if not isinstance(alloc, mybir.MemoryLocationSet):
            continue
        name = alloc.memorylocations[0].name
        if alloc.kind == "ExternalInput":
            if name != partition_name:
                in_names.append(name)
        elif alloc.kind == "ExternalOutput":
            shape = tuple(alloc.tensor_shape)
            dtype = mybir.dt.np(alloc.dtype)
            out_names.append(name)
            out_avals.append(jax.core.ShapedArray(shape, dtype))
            zero_specs.append((shape, dtype))
    n_params = len(in_names)
    all_names = list(in_names) + out_names
    if partition_name is not None:
        all_names.append(partition_name)
    donate = tuple(range(n_params, n_params + len(out_names)))

    def _body(*args):
        operands = list(args)
        if partition_name is not None:
            operands.append(bass2jax.partition_id_tensor())
        return tuple(bass2jax._bass_exec_p.bind(
            *operands,
            out_avals=tuple(out_avals),
            in_names=tuple(all_names),
            out_names=tuple(out_names),
            lowering_input_output_aliases=(),
            sim_require_finite=True,
            sim_require_nnan=True,
            nc=nc,
        ))

    devices = jax.devices()[:NCORES]
    mesh = Mesh(np.asarray(devices), ("core",))
    nin = n_params + len(out_names)
    sharded = jax.jit(
        shard_map(_body, mesh=mesh, in_specs=(PartitionSpec("core"),) * nin,
                  out_specs=(PartitionSpec("core"),) * len(out_names),
                  check_rep=False),
        donate_argnums=donate, keep_unused=True)
    iy = out_names.index("y")

    def run(in_maps):
        concat_in = [
            np.concatenate([np.asarray(m[n]) for m in in_maps], axis=0)
            for n in in_names
        ]
        concat_zeros = [
            np.zeros((NCORES * s[0], *s[1:]), d) for (s, d) in zero_specs
        ]
        out = sharded(*concat_in, *concat_zeros)
        return np.asarray(out[iy])[:G]

    return run


def kernel(**inputs):
    x = np.asarray(inputs["x"]).astype(np.int64).reshape(-1)
    ei = np.asarray(inputs["edge_index"]).astype(np.int64)
    batch = np.asarray(inputs["batch"]).astype(np.int64).reshape(-1)
    emb = np.asarray(inputs["emb"], np.float32)
    W1 = np.asarray(inputs["W1"], np.float32)
    b1 = np.asarray(inputs["b1"], np.float32).reshape(1, -1)
    W2 = np.asarray(inputs["W2"], np.float32)
    b2 = np.asarray(inputs["b2"], np.float32).reshape(1, -1)
    Wc1 = np.asarray(inputs["Wc1"], np.float32)
    bc1 = np.asarray(inputs["bc1"], np.float32).reshape(1, -1)
    Wc2 = np.asarray(inputs["Wc2"], np.float32)
    bc2 = np.asarray(inputs["bc2"], np.float32).reshape(1, -1)

    plan, nslots, per_core = _shard_edges(ei[0], ei[1])

    key = (nslots, tuple((s, d, lo, hi, tuple(sp)) for (s, d, lo, hi, sp) in plan))
    if key not in _PROG_CACHE:
        _PROG_CACHE[key] = _build_program(plan, nslots)
        _RUN_CACHE[key] = _make_runner(_PROG_CACHE[key])
    nc = _PROG_CACHE[key]

    # in-degree incl. self-loop, laid out in padded row space for the c2 load
    deg = np.bincount(ei[1], minlength=N).astype(np.float32) + 1.0
    deg_pad = np.zeros(NROW, np.float32)
    rows_all = (np.arange(N) // CH_REAL) * CH_PAD + (np.arange(N) % CH_REAL)
    deg_pad[rows_all] = deg

    # node-id table in padded row space (row = (n//CH_REAL)*CH_PAD + n%CH_REAL)
    ids_pad = np.zeros(NROW, np.int64)
    rows = (np.arange(N) // CH_REAL) * CH_PAD + (np.arange(N) % CH_REAL)
    ids_pad[rows] = x
    ids16 = _wrap16(ids_pad.astype(np.int16))

    # batch values for each core's final-phase rows: core c, chunk d,
    # subtile s, partition p -> chunk-local row FSLICE*c + s*128 + p
    iota256 = np.tile(np.arange(G, dtype=np.float32), (128, 1))
    ident128 = np.eye(128, dtype=np.float32)

    loc = np.arange(FSLICE)
    in_maps = []
    for c in range(NCORES):
        fl = FSLICE * c + loc          # chunk-local rows this core handles
        fidx = _wrap16(fl.astype(np.int16))
        bvals = np.full((NCHUNK, FSLICE), -1.0, np.float32)
        for d in range(NCHUNK):
            gl = fl.copy()
            real = gl < CH_REAL
            n_global = d * CH_REAL + gl
            ok = real & (n_global < N)
            bvals[d, ok] = batch[n_global[ok]]
        # [128, NCHUNK*FSLICE/128] with col d*NS + s at partition p = row s*128+p
        barr = bvals.reshape(NCHUNK, FSLICE // 128, 128).transpose(2, 0, 1).reshape(
            128, NCHUNK * (FSLICE // 128))
        in_maps.append(dict(
            ids16=ids16,
            batchf=np.ascontiguousarray(barr),
            gidx=_wrap16(per_core[c]["gidx"]),
            sidx=_wrap16(per_core[c]["sidx"]),
            fidx=fidx,
            emb=emb, W1=W1, b1=b1, W2=W2, b2=b2,
            Wc1=Wc1, bc1=bc1, Wc2=Wc2, bc2=bc2,
            iota256=iota256, ident128=ident128, degf=deg_pad,
        ))

    return _RUN_CACHE[key](in_maps).astype(np.float32)

